# revision 1
# baseline (speedup 1.0000x reference)
"""Bass/Tile kernel builder for distributed causal MHA with RoPE on 8 NeuronCores.

Sharding: head-pair per core (16 heads / 8 cores = 2 heads each), both batches
on every core.  After attention, one 8-core AllToAll redistributes the per-head
context so core c assembles the full context for (batch c//4, seq-quarter c%4)
and applies the output projection locally.  Host concatenates the 8 quarters.

All matmuls run as float32r (FP22-truncated fp32) at full PE rate.
"""

import sys

sys.path.insert(0, "/opt/trn_rl_repo")

import numpy as np
try:
    from ml_dtypes import bfloat16 as np_bf16
except ImportError:
    import jax.numpy as _jnp
    np_bf16 = _jnp.bfloat16
import concourse.bass as bass
import concourse.mybir as mybir
import concourse.tile as tile
from concourse import bacc
from concourse.masks import make_identity

F32 = mybir.dt.float32
F32R = mybir.dt.float32r
BF16 = mybir.dt.bfloat16

D_MODEL = 1024
NUM_HEADS = 16
DHEAD = 64
THETA = 10000.0
N_CORES = 8
B = 2


def r(ap):
    """bitcast an fp32 AP to float32r for matmul operands."""
    return ap.bitcast(F32R)


def build_nc(S, single_core=False, reps=1):
    """Build the SPMD Bass program (identical on all 8 cores)."""
    assert S % 512 == 0
    SQ = S // 4            # seq quarter each core outputs
    NJ = S // 512          # number of 512-wide sq chunks
    NK = S // 128          # number of 128-tall sk tiles
    CW = min(512, SQ)      # chunk width in the Wo phase
    NC2 = SQ // CW         # chunks per quarter
    NST = SQ // 128        # 128-row out tiles per quarter

    nc = bacc.Bacc("TRN2", target_bir_lowering=False, debug=False,
                   num_devices=1 if single_core else N_CORES)

    # ---- I/O ----
    xt = nc.dram_tensor("xt", [B, D_MODEL, S], BF16, kind="ExternalInput")
    wq = nc.dram_tensor("wq", [D_MODEL, 128], BF16, kind="ExternalInput")
    wk = nc.dram_tensor("wk", [D_MODEL, 128], BF16, kind="ExternalInput")
    wv = nc.dram_tensor("wv", [D_MODEL, 128], BF16, kind="ExternalInput")
    wo = nc.dram_tensor("wo", [D_MODEL, D_MODEL], F32, kind="ExternalInput")
    cosm = nc.dram_tensor("cosm", [128, S], BF16, kind="ExternalInput")
    sinm = nc.dram_tensor("sinm", [128, S], BF16, kind="ExternalInput")
    sel2 = nc.dram_tensor("sel2", [2, 128], F32, kind="ExternalInput")
    out = nc.dram_tensor("out", [SQ, D_MODEL], F32, kind="ExternalOutput")

    import contextlib
    with tile.TileContext(nc) as tc:
        rep_loop = (tc.For_i(0, reps, 1) if reps > 1
                    else contextlib.nullcontext())
        with (
            rep_loop,
            tc.tile_pool(name="persist", bufs=1) as pp,
            tc.tile_pool(name="dram", bufs=1, space="DRAM") as dram,
        ):
            qp = tc.alloc_tile_pool(name="qkv", bufs=1)
            # long-lived sbuf tensors (released after attention)
            qt = [qp.tile([128, S], BF16, name=f"qt{b}") for b in range(B)]
            kt = [qp.tile([128, S], BF16, name=f"kt{b}") for b in range(B)]
            vsb = [[qp.tile([128, 130], BF16, name=f"v{b}_{st}")
                    for st in range(NK)] for b in range(B)]
            sel2_sb = pp.tile([2, 128], F32R, name="sel2_sb")
            nc.sync.dma_start(sel2_sb[:], r(sel2[:]))
            onesc = pp.tile([128, 2], BF16, name="onesc")
            nc.vector.memset(onesc[:], 1.0)
            cos_sb = pp.tile([128, S], BF16, name="cos_sb")
            sin_sb = pp.tile([128, S], BF16, name="sin_sb")
            nc.sync.dma_start(cos_sb[:], cosm[:])
            nc.sync.dma_start(sin_sb[:], sinm[:])
            rp2 = tc.alloc_tile_pool(name="rope2", bufs=2)

            # ---------------- Phase 1: projections ----------------
            with (
                tc.tile_pool(name="wts", bufs=1) as wp,
                tc.tile_pool(name="xch", bufs=2) as xp,
                tc.tile_pool(name="p1ps", bufs=2, space="PSUM") as ps1,
                tc.tile_pool(name="vps", bufs=2, space="PSUM") as psv,
            ):
                wq_sb = wp.tile([128, 8, 128], BF16, name="wq_sb")
                wk_sb = wp.tile([128, 8, 128], BF16, name="wk_sb")
                wv_sb = wp.tile([128, 8, 128], BF16, name="wv_sb")
                ident = wp.tile([128, 128], BF16, name="ident")
                make_identity(nc, ident[:])
                for kk in range(8):
                    nc.sync.dma_start(wq_sb[:, kk, :], wq[128 * kk:128 * kk + 128, :])
                    nc.sync.dma_start(wk_sb[:, kk, :], wk[128 * kk:128 * kk + 128, :])
                    nc.sync.dma_start(wv_sb[:, kk, :], wv[128 * kk:128 * kk + 128, :])

                for b in range(B):
                    for sc in range(NJ):
                        s0 = 512 * sc
                        xch = xp.tile([128, 8, 512], BF16, name="xch", tag="xch")
                        for kk in range(8):
                            nc.sync.dma_start(
                                xch[:, kk, :],
                                xt[b, 128 * kk:128 * kk + 128, s0:s0 + 512])
                        q_ps = ps1.tile([128, 512], F32, name="q_ps", tag="q")
                        k_ps = ps1.tile([128, 512], F32, name="k_ps", tag="k")
                        vt_ps = ps1.tile([128, 512], F32, name="vt_ps", tag="vt")
                        for kk in range(8):
                            nc.tensor.matmul(q_ps[:], wq_sb[:, kk, :],
                                             xch[:, kk, :],
                                             start=(kk == 0), stop=(kk == 7))
                        for kk in range(8):
                            nc.tensor.matmul(k_ps[:], wk_sb[:, kk, :],
                                             xch[:, kk, :],
                                             start=(kk == 0), stop=(kk == 7))
                        for kk in range(8):
                            nc.tensor.matmul(vt_ps[:], wv_sb[:, kk, :],
                                             xch[:, kk, :],
                                             start=(kk == 0), stop=(kk == 7))
                        nc.vector.tensor_copy(qt[b][:, s0:s0 + 512], q_ps[:])
                        nc.vector.tensor_copy(kt[b][:, s0:s0 + 512], k_ps[:])
                        for ten in (qt[b], kt[b]):
                            sl_ = slice(s0, s0 + 512)
                            t1_ = rp2.tile([128, 512], BF16, name="t1", tag="t1")
                            t2_ = rp2.tile([128, 512], BF16, name="t2", tag="t2")
                            t2s_ = rp2.tile([128, 512], BF16, name="t2s",
                                            tag="t2s")
                            nc.vector.tensor_mul(t1_[:], ten[:, sl_],
                                                 cos_sb[:, sl_])
                            nc.vector.tensor_mul(t2_[:], ten[:, sl_],
                                                 sin_sb[:, sl_])
                            for blk in range(4):
                                src2 = 32 * (blk ^ 1)
                                nc.sync.dma_start(
                                    t2s_[32 * blk:32 * blk + 32, :],
                                    t2_[src2:src2 + 32, :])
                            nc.vector.tensor_add(ten[:, sl_], t1_[:], t2s_[:])
                        vt_sb = xp.tile([128, 512], BF16, name="vt_sb", tag="vtsb")
                        nc.vector.tensor_copy(vt_sb[:], vt_ps[:])
                        # transpose [m, s] -> [s, m] per 128-block via PE
                        for st in range(4):
                            v_ps = psv.tile([128, 128], BF16, name="v_ps", tag="v")
                            nc.tensor.transpose(
                                v_ps[:], vt_sb[:, 128 * st:128 * st + 128],
                                ident[:])
                            vt = vsb[b][4 * sc + st]
                            # layout [V_h0 | 1 | V_h1 | 1]: ones at cols 64, 129
                            vt3 = vt[:].rearrange("p (a b) -> p a b", a=2)
                            nc.vector.tensor_copy(
                                vt3[:, :, 64:65],
                                onesc[:].rearrange("p (a b) -> p a b", a=2))
                            nc.vector.tensor_copy(
                                vt3[:, :, 0:64],
                                v_ps[:].rearrange("p (a b) -> p a b", a=2))

            # -------- Phase 2+3: rope overlapped with attention --------
            ib = dram.tile([8, 130, SQ], F32, name="ib")
            ob = dram.tile([8, 130, SQ], F32, name="ob")
            with (
                tc.tile_pool(name="tables", bufs=1) as tabp,
                tc.tile_pool(name="ropetmp", bufs=2) as rp,
                tc.tile_pool(name="scps", bufs=2, space="PSUM") as scp,
                tc.tile_pool(name="avps", bufs=1, space="PSUM") as avp,
                tc.tile_pool(name="ptp", bufs=3) as ptp,
                tc.tile_pool(name="cxp", bufs=4) as cxp,
            ):
                for b in range(B):
                    for j in range(NJ):
                        nk = min(4 * j + 4, NK)
                        q0 = 512 * j
                        nslot = 2 * nk
                        ngroup = (nslot + 2) // 3
                        sc_t = [scp.tile([128, 1536], F32, name="sc_t", tag="sc")
                                for _ in range(ngroup)]
                        pt_t = [ptp.tile([128, 1536], BF16, name="pt_t", tag="pt")
                                for _ in range(ngroup)]

                        def slot_ap(tiles, s):
                            return tiles[s // 3][:, 512 * (s % 3):512 * (s % 3) + 512]

                        # scores + exp
                        for k in range(nk):
                            for h in range(2):
                                s = 2 * k + h
                                hb = 64 * h
                                nc.tensor.matmul(
                                    slot_ap(sc_t, s),
                                    kt[b][hb:hb + 64, 128 * k:128 * k + 128],
                                    qt[b][hb:hb + 64, q0:q0 + 512],
                                    start=True, stop=True)
                            # when a group of 3 fills (or last slot), exp it
                        for g in range(ngroup):
                            w = min(1536, (nslot - 3 * g) * 512)
                            nc.scalar.activation(pt_t[g][:, 0:w], sc_t[g][:, 0:w],
                                                 mybir.ActivationFunctionType.Exp,
                                                 scale=0.125)
                        # causal mask on band tiles (k in [4j, 4j+3])
                        for k in range(max(0, 4 * j), nk):
                            base = 512 * j - 128 * k
                            for h in range(2):
                                s = 2 * k + h
                                ap = slot_ap(pt_t, s)
                                nc.gpsimd.affine_select(
                                    ap, ap, pattern=[[1, 512]],
                                    compare_op=mybir.AluOpType.is_ge,
                                    fill=0.0, base=base, channel_multiplier=-1)
                        # AV: interleave both heads' accumulation chains so pt
                        # groups retire in slot order (lets the pt pool recycle)
                        av = [avp.tile([65, 512], F32, name=f"av{h}", tag=f"av{h}")
                              for h in range(2)]
                        for k in range(nk):
                            for h in range(2):
                                nc.tensor.matmul(
                                    av[h][:], vsb[b][k][:, 65 * h:65 * h + 65],
                                    slot_ap(pt_t, 2 * k + h),
                                    start=(k == 0), stop=(k == nk - 1))
                        # drain ctx+denom to SBUF (partition-aligned), recip the
                        # denom, then DMA straight into the A2A input bounce
                        for h in range(2):
                            cx = cxp.tile([65, 512], F32, name="cx", tag="cx")
                            nc.vector.tensor_copy(cx[:], av[h][:])
                            nc.vector.reciprocal(cx[64:65, :], cx[64:65, :])
                            # split the 512-chunk by seq-quarter boundaries
                            c0 = q0
                            while c0 < q0 + 512:
                                g2 = c0 // SQ
                                w = min(SQ * (g2 + 1), q0 + 512) - c0
                                j2 = b * 4 + g2
                                lo, li = c0 - SQ * g2, c0 - q0
                                nc.sync.dma_start(
                                    ib[j2, 0 + 64 * h:64 * h + 64, lo:lo + w],
                                    cx[0:64, li:li + w])
                                nc.sync.dma_start(
                                    ib[j2, 128 + h:129 + h, lo:lo + w],
                                    cx[64:65, li:li + w])
                                c0 += w

            rp2.release()
            qp.release()

            # ---------------- Phase 4: A2A + output projection ----------------
            if single_core or reps > 1:
                nc.gpsimd.dma_start(ob[:], ib[:])
            else:
                nc.gpsimd.collective_compute(
                    "AllToAll", mybir.AluOpType.bypass,
                    replica_groups=[list(range(8))],
                    ins=[ib.opt()], outs=[ob.opt()])

            with (
                tc.tile_pool(name="wophase", bufs=1) as wop,
                tc.tile_pool(name="ctxsp", bufs=1) as csp,
                tc.tile_pool(name="wops", bufs=2, space="PSUM") as wops,
                tc.tile_pool(name="bcps", bufs=2, space="PSUM") as bcps,
                tc.tile_pool(name="osbp", bufs=3) as osbp,
            ):
                wo_sb = wop.tile([128, 8, D_MODEL], F32R, name="wo_sb")
                for t in range(8):
                    nc.sync.dma_start(wo_sb[:, t, :], r(wo[128 * t:128 * t + 128, :]))
                ctxs = []
                for t in range(8):
                    ctxf = wop.tile([128, SQ], F32, name=f"ctxf{t}")
                    rq = wop.tile([2, SQ], F32R, name=f"rq{t}")
                    nc.sync.dma_start(ctxf[:], ob[t, 0:128, :])
                    nc.sync.dma_start(rq[:], r(ob[t, 128:130, :]))
                    row = []
                    for c2 in range(NC2):
                        cl = slice(CW * c2, CW * (c2 + 1))
                        bc = bcps.tile([128, CW], F32, name="bc", tag="bc")
                        nc.tensor.matmul(bc[:], r(sel2_sb[:]), r(rq[:, cl]),
                                         start=True, stop=True)
                        cst = csp.tile([128, CW], F32R, name=f"ctxs{t}_{c2}")
                        nc.vector.tensor_mul(cst[:], ctxf[:, cl], bc[:])
                        row.append(cst)
                    ctxs.append(row)
                for st in range(NST):
                    for m2 in range(2):
                        wo_ps = wops.tile([128, 512], F32, name="wo_ps", tag="wo")
                        for t in range(8):
                            cst = ctxs[t][(128 * st) // CW]
                            coff = (128 * st) % CW
                            nc.tensor.matmul(
                                wo_ps[:], r(cst[:, coff:coff + 128]),
                                r(wo_sb[:, t, 512 * m2:512 * m2 + 512]),
                                start=(t == 0), stop=(t == 7))
                        osb = osbp.tile([128, 512], F32, name="osb", tag="osb")
                        nc.vector.tensor_copy(osb[:], wo_ps[:])
                        nc.sync.dma_start(
                            out[128 * st:128 * st + 128, 512 * m2:512 * m2 + 512],
                            osb[:])

    nc.compile()
    return nc


# ---------------------------------------------------------------------------
# Host-side sharding / assembly
# ---------------------------------------------------------------------------

def _rope_tables(token_positions, S):
    half = DHEAD // 2
    inv_freq = THETA ** (-2.0 * np.arange(half, dtype=np.float32) / DHEAD)
    angles = np.arange(4096, dtype=np.float32)[:, None] * inv_freq[None, :]
    cos_c, sin_c = np.cos(angles), np.sin(angles)
    pos = np.asarray(token_positions).astype(np.int64)
    cosT = cos_c[pos].T.astype(np.float32)   # [32, S]
    sinT = sin_c[pos].T.astype(np.float32)
    cosm = np.concatenate([cosT, cosT, cosT, cosT], 0)
    sinm = np.concatenate([sinT, -sinT, sinT, -sinT], 0)
    return (np.ascontiguousarray(cosm).astype(np_bf16),
            np.ascontiguousarray(sinm).astype(np_bf16))


def prepare_in_maps(in_features, token_positions, Wq, Wk, Wv, Wo):
    Bb, S, D = in_features.shape
    xt = np.ascontiguousarray(in_features.transpose(0, 2, 1)).astype(np_bf16)
    cosm, sinm = _rope_tables(token_positions, S)
    sel2 = np.zeros((2, 128), np.float32)
    sel2[0, :64] = 1.0
    sel2[1, 64:] = 1.0
    perm = np.concatenate([np.arange(0, 64, 2), np.arange(1, 64, 2)])
    woT = np.ascontiguousarray(Wo.T).astype(np.float32)
    in_maps = []
    for c in range(N_CORES):
        h0, h1 = 2 * c, 2 * c + 1
        blocks_qk = []
        for W in (Wq, Wk):
            cols = []
            for h in (h0, h1):
                blk = W[64 * h:64 * h + 64, :][perm, :]   # [64, D] permuted
                cols.append(blk.T)                         # [D, 64]
            blocks_qk.append(np.ascontiguousarray(
                np.concatenate(cols, axis=1)).astype(np_bf16))
        wv_c = np.ascontiguousarray(np.concatenate(
            [Wv[64 * h:64 * h + 64, :].T for h in (h0, h1)],
            axis=1)).astype(np_bf16)
        in_maps.append({
            "xt": xt, "wq": blocks_qk[0], "wk": blocks_qk[1], "wv": wv_c,
            "wo": woT, "cosm": cosm, "sinm": sinm, "sel2": sel2,
        })
    return in_maps


def assemble(results, S):
    SQ = S // 4
    out = np.zeros((B, S, D_MODEL), np.float32)
    for c in range(N_CORES):
        b, g = c // 4, c % 4
        out[b, SQ * g:SQ * (g + 1), :] = results[c]["out"]
    return out

from concourse.bass_utils import run_bass_kernel_spmd

_S = 4096
_NC = None


def _get_nc():
    global _NC
    if _NC is None:
        _NC = build_nc(_S)
    return _NC


def kernel(in_features, token_positions, Wq, Wk, Wv, Wo):
    x = np.asarray(in_features, dtype=np.float32)
    pos = np.asarray(token_positions)
    Wq = np.asarray(Wq, dtype=np.float32)
    Wk = np.asarray(Wk, dtype=np.float32)
    Wv = np.asarray(Wv, dtype=np.float32)
    Wo = np.asarray(Wo, dtype=np.float32)
    nc = _get_nc()
    in_maps = prepare_in_maps(x, pos, Wq, Wk, Wv, Wo)
    res = run_bass_kernel_spmd(nc, in_maps, list(range(N_CORES)))
    return assemble(res.results, _S)

from concourse.bass_utils import run_bass_kernel_spmd

_S = 4096
_NC = None


def _get_nc():
    global _NC
    if _NC is None:
        _NC = build_nc(_S)
    return _NC


def kernel(in_features, token_positions, Wq, Wk, Wv, Wo):
    x = np.asarray(in_features, dtype=np.float32)
    pos = np.asarray(token_positions)
    Wq = np.asarray(Wq, dtype=np.float32)
    Wk = np.asarray(Wk, dtype=np.float32)
    Wv = np.asarray(Wv, dtype=np.float32)
    Wo = np.asarray(Wo, dtype=np.float32)
    nc = _get_nc()
    in_maps = prepare_in_maps(x, pos, Wq, Wk, Wv, Wo)
    res = run_bass_kernel_spmd(nc, in_maps, list(range(N_CORES)))
    return assemble(res.results, _S)



# revision 2
# speedup vs baseline: 1.0802x; 1.0802x over previous
"""Bass/Tile kernel builder for distributed causal MHA with RoPE on 8 NeuronCores.

v2: per-batch pipelined A2A.  Head-pair per core (16 heads / 8 cores), both
batches on every core.  Core c assembles the full context for tokens
[512c, 512c+512) of EACH batch, so attention chunk j maps 1:1 to destination
core j and the AllToAll can run per batch: batch-0's A2A and output
projection overlap batch-1's projection/attention.  A2A payload is bf16.

All matmuls run bf16 at full PE rate.
"""

import sys

sys.path.insert(0, "/opt/trn_rl_repo")

import numpy as np
try:
    from ml_dtypes import bfloat16 as np_bf16
except ImportError:
    import jax.numpy as _jnp
    np_bf16 = _jnp.bfloat16
import concourse.bass as bass
import concourse.mybir as mybir
import concourse.tile as tile
from concourse import bacc
from concourse.masks import make_identity

F32 = mybir.dt.float32
F32R = mybir.dt.float32r
BF16 = mybir.dt.bfloat16

D_MODEL = 1024
NUM_HEADS = 16
DHEAD = 64
THETA = 10000.0
N_CORES = 8
B = 2


def build_nc(S, single_core=False, reps=1):
    """Build the SPMD Bass program (identical on all 8 cores)."""
    assert S == 4096
    CH = S // 8            # tokens per core per batch (512)
    NJ = S // 512          # number of 512-wide chunks per batch (8)
    NK = S // 128          # number of 128-tall sk tiles (32)

    import os
    rope_pe = os.environ.get("KROPE", "dma") == "pe"
    mask_dve = os.environ.get("KMASK", "dve") == "dve"
    nc = bacc.Bacc("TRN2", target_bir_lowering=False, debug=False,
                   num_devices=1 if single_core else N_CORES)

    # ---- I/O ----
    xt = nc.dram_tensor("xt", [B, D_MODEL, S], BF16, kind="ExternalInput")
    wq = nc.dram_tensor("wq", [D_MODEL, 128], BF16, kind="ExternalInput")
    wk = nc.dram_tensor("wk", [D_MODEL, 128], BF16, kind="ExternalInput")
    wv = nc.dram_tensor("wv", [D_MODEL, 128], BF16, kind="ExternalInput")
    wo = nc.dram_tensor("wo", [D_MODEL, D_MODEL], BF16, kind="ExternalInput")
    cosm = nc.dram_tensor("cosm", [128, S], BF16, kind="ExternalInput")
    sinm = nc.dram_tensor("sinm", [128, S], BF16, kind="ExternalInput")
    sel2 = nc.dram_tensor("sel2", [2, 128], BF16, kind="ExternalInput")
    trimask = nc.dram_tensor("trimask", [128, 8, 512], BF16,
                             kind="ExternalInput")
    sel16 = nc.dram_tensor("sel16", [16, 8, 128], BF16, kind="ExternalInput")
    # rows [0:512) = batch-0 tokens [512c, 512c+512), rows [512:1024) batch-1
    out = nc.dram_tensor("out", [2 * CH, D_MODEL], F32, kind="ExternalOutput")

    import contextlib
    with tile.TileContext(nc) as tc:
        rep_loop = (tc.For_i(0, reps, 1) if reps > 1
                    else contextlib.nullcontext())
        with (
            rep_loop,
            tc.tile_pool(name="persist", bufs=1) as pp,
            tc.tile_pool(name="dram", bufs=1, space="DRAM") as dram,
        ):
            # ---- persistent tiles + prefetches ----
            # weights first on the SP queue (first matmuls wait on them);
            # bulk tables go via the Pool queue
            wq_sb = pp.tile([128, 8, 128], BF16, name="wq_sb")
            wk_sb = pp.tile([128, 8, 128], BF16, name="wk_sb")
            wv_sb = pp.tile([128, 8, 128], BF16, name="wv_sb")
            nc.sync.dma_start(wq_sb[:], wq.rearrange("(g p) d -> p g d", p=128))
            nc.sync.dma_start(wk_sb[:], wk.rearrange("(g p) d -> p g d", p=128))
            nc.sync.dma_start(wv_sb[:], wv.rearrange("(g p) d -> p g d", p=128))
            sel2_sb = pp.tile([2, 128], BF16, name="sel2_sb")
            nc.gpsimd.dma_start(sel2_sb[:], sel2[:])
            onesc = pp.tile([128, 2], BF16, name="onesc")
            nc.vector.memset(onesc[:], 1.0)
            cos_sb = pp.tile([128, S], BF16, name="cos_sb")
            sin_sb = pp.tile([128, S], BF16, name="sin_sb")
            nc.gpsimd.dma_start(cos_sb[:], cosm[:])
            nc.gpsimd.dma_start(sin_sb[:], sinm[:])
            ident = pp.tile([128, 128], BF16, name="ident")
            make_identity(nc, ident[:])
            tm_sb = pp.tile([128, 8, 512], BF16, name="tm_sb")
            nc.gpsimd.dma_start(tm_sb[:], trimask[:])
            sel16_sb = pp.tile([16, 8, 128], BF16, name="sel16_sb")
            nc.gpsimd.dma_start(sel16_sb[:], sel16[:])
            # SWAP permutation (32-block partner swap) built from ident
            swap = pp.tile([128, 128], BF16, name="swap")
            for blk in range(4):
                src2 = 32 * (blk ^ 1)
                nc.gpsimd.dma_start(swap[32 * blk:32 * blk + 32, :],
                                    ident[src2:src2 + 32, :])
            wo_sb = pp.tile([128, 8, D_MODEL], BF16, name="wo_sb")
            nc.gpsimd.dma_start(wo_sb[:],
                                wo.rearrange("(g p) d -> p g d", p=128))

            # long-lived per-chunk q/k tiles and v tiles
            qp = tc.alloc_tile_pool(name="qkv", bufs=1)
            qt = [[qp.tile([128, 512], BF16, name=f"qt{b}_{j}")
                   for j in range(NJ)] for b in range(B)]
            kt = [[qp.tile([128, 512], BF16, name=f"kt{b}_{j}")
                   for j in range(NJ)] for b in range(B)]
            vsb = [[qp.tile([128, 130], BF16, name=f"v{b}_{st}")
                    for st in range(NK)] for b in range(B)]
            rp2 = tc.alloc_tile_pool(name="rope2", bufs=2)

            # per-batch A2A bounce buffers (bf16)
            # chunk layout: rows [0:65) = h0 ctx+denom, [65:130) = h1
            ib = [dram.tile([8, 130, CH], BF16, name=f"ib{b}") for b in range(B)]
            ob = [dram.tile([8, 130, CH], BF16, name=f"ob{b}") for b in range(B)]

            def proj_chunk(b, sc, xp, ps1, psv):
                """QKV projection + RoPE for one 512-token chunk."""
                s0 = 512 * sc
                xch = xp.tile([128, 8, 512], BF16, name="xch", tag="xch")
                nc.sync.dma_start(
                    xch[:],
                    xt[b, :, s0:s0 + 512].rearrange("(g p) s -> p g s", p=128))
                for ten, wsb in ((qt[b][sc], wq_sb), (kt[b][sc], wk_sb)):
                    ps = ps1.tile([128, 512], F32, name="pps", tag="rot")
                    for kk in range(8):
                        nc.tensor.matmul(ps[:], wsb[:, kk, :], xch[:, kk, :],
                                         start=(kk == 0), stop=(kk == 7))
                    nc.vector.tensor_copy(ten[:], ps[:])
                    t1_ = rp2.tile([128, 512], BF16, name="t1", tag="t1")
                    t2_ = rp2.tile([128, 512], BF16, name="t2", tag="t2")
                    t2s_ = rp2.tile([128, 512], BF16, name="t2s", tag="t2s")
                    nc.vector.tensor_mul(t1_[:], ten[:], cos_sb[:, s0:s0 + 512])
                    nc.vector.tensor_mul(t2_[:], ten[:], sin_sb[:, s0:s0 + 512])
                    for blk in range(4):
                        src2 = 32 * (blk ^ 1)
                        nc.gpsimd.dma_start(
                            t2s_[32 * blk:32 * blk + 32, :],
                            t2_[src2:src2 + 32, :])
                    nc.vector.tensor_add(ten[:], t1_[:], t2s_[:])
                vt_ps = ps1.tile([128, 512], F32, name="pps", tag="rot")
                for kk in range(8):
                    nc.tensor.matmul(vt_ps[:], wv_sb[:, kk, :], xch[:, kk, :],
                                     start=(kk == 0), stop=(kk == 7))
                vt_sb = xp.tile([128, 512], BF16, name="vt_sb", tag="vtsb")
                nc.vector.tensor_copy(vt_sb[:], vt_ps[:])
                for st in range(4):
                    v_ps = psv.tile([128, 128], BF16, name="v_ps", tag="v")
                    nc.tensor.transpose(
                        v_ps[:], vt_sb[:, 128 * st:128 * st + 128], ident[:])
                    vt = vsb[b][4 * sc + st]
                    vt3 = vt[:].rearrange("p (a b) -> p a b", a=2)
                    nc.vector.tensor_copy(
                        vt3[:, :, 64:65],
                        onesc[:].rearrange("p (a b) -> p a b", a=2))
                    nc.vector.tensor_copy(
                        vt3[:, :, 0:64],
                        v_ps[:].rearrange("p (a b) -> p a b", a=2))

            SPG = 2  # score-group width in 512-slots (PSUM: 2 banks/group)

            def attn_chunk(b, j, scp, avp, ptp, cxp):
                """Causal attention for one 512-query chunk; writes ib[b][j]."""
                nk = min(4 * j + 4, NK)
                nslot = 2 * nk
                ngroup = (nslot + SPG - 1) // SPG
                sc_t = [scp.tile([128, 512 * SPG], F32, name="sc_t", tag="sc")
                        for _ in range(ngroup)]
                pt_t = [ptp.tile([128, 512 * SPG], BF16, name="pt_t", tag="pt")
                        for _ in range(ngroup)]

                def slot_ap(tiles, s):
                    o = 512 * (s % SPG)
                    return tiles[s // SPG][:, o:o + 512]

                for k in range(nk):
                    for h in range(2):
                        s = 2 * k + h
                        hb = 64 * h
                        nc.tensor.matmul(
                            slot_ap(sc_t, s),
                            kt[b][k // 4][hb:hb + 64,
                                          128 * (k % 4):128 * (k % 4) + 128],
                            qt[b][j][hb:hb + 64, :],
                            start=True, stop=True)
                for g in range(ngroup):
                    w = min(512 * SPG, (nslot - SPG * g) * 512)
                    nc.scalar.activation(pt_t[g][:, 0:w], sc_t[g][:, 0:w],
                                         mybir.ActivationFunctionType.Exp,
                                         scale=0.125)
                # causal mask: band = last 8 slots, grouped muls
                s_lo = nslot - 8
                for g in range(s_lo // SPG, ngroup):
                    a = max(SPG * g, s_lo)
                    z = min(SPG * g + SPG, nslot)
                    o = 512 * (a % SPG)
                    nc.vector.tensor_mul(
                        pt_t[g][:, o:o + 512 * (z - a)],
                        pt_t[g][:, o:o + 512 * (z - a)],
                        tm_sb[:, a - s_lo:z - s_lo, :])
                av = [avp.tile([65, 512], F32, name=f"av{h}", tag=f"av{h}")
                      for h in range(2)]
                for k in range(nk):
                    for h in range(2):
                        nc.tensor.matmul(
                            av[h][:], vsb[b][k][:, 65 * h:65 * h + 65],
                            slot_ap(pt_t, 2 * k + h),
                            start=(k == 0), stop=(k == nk - 1))
                for h in range(2):
                    cx = cxp.tile([65, 512], BF16, name="cx", tag="cx")
                    nc.vector.tensor_copy(cx[:], av[h][:])
                    nc.sync.dma_start(
                        ib[b][j, 65 * h:65 * h + 65, :], cx[:])
                if single_core or reps > 1:
                    # timed-mode A2A substitute, priced per chunk
                    nc.gpsimd.dma_start(ob[b][j], ib[b][j])

            def a2a(b):
                if single_core or reps > 1:
                    pass  # per-chunk copies emitted in attn_chunk
                else:
                    nc.gpsimd.collective_compute(
                        "AllToAll", mybir.AluOpType.bypass,
                        replica_groups=[list(range(8))],
                        ins=[ib[b].opt()], outs=[ob[b].opt()])

            def wo_pieces(b, wop, wops, bcps, osbp):
                """Output projection for batch b as a list of emitters."""
                state = {}
                pieces = []

                def recip_piece():
                    rqall = wop.tile([16, CH], BF16, name=f"rqall{b}")
                    nc.sync.dma_start(
                        rqall[:],
                        ob[b][:].rearrange("c (a r) s -> (c a) r s", a=2)[
                            :, 64, :])
                    with nc.allow_low_precision(
                            reason="bf16 1/denom, 0.4% rel err ok"):
                        nc.vector.reciprocal(rqall[:], rqall[:])
                    state["rqall"] = rqall
                    state["csts"] = []
                pieces.append(recip_piece)

                def ctx_piece(t):
                    ctxf = wop.tile([128, CH], BF16, name=f"ctxf{b}_{t}")
                    nc.sync.dma_start(ctxf[0:64, :], ob[b][t, 0:64, :])
                    nc.sync.dma_start(ctxf[64:128, :], ob[b][t, 65:129, :])
                    bc = bcps.tile([128, CH], F32, name="bc", tag="bc")
                    nc.tensor.matmul(bc[:], sel16_sb[:, t, :],
                                     state["rqall"][:], start=True, stop=True)
                    cst = wop.tile([128, CH], BF16, name=f"cst{b}_{t}")
                    nc.vector.tensor_mul(cst[:], ctxf[:], bc[:])
                    state["csts"].append(cst)
                for t in range(8):
                    pieces.append(lambda t=t: ctx_piece(t))

                def st_piece(st):
                    osb = osbp.tile([128, D_MODEL], F32, name="osb", tag="osb")
                    for m2 in range(2):
                        wo_ps = wops.tile([128, 512], F32, name="wo_ps", tag="wo")
                        for t in range(8):
                            nc.tensor.matmul(
                                wo_ps[:],
                                state["csts"][t][:, 128 * st:128 * st + 128],
                                wo_sb[:, t, 512 * m2:512 * m2 + 512],
                                start=(t == 0), stop=(t == 7))
                        nc.vector.tensor_copy(
                            osb[:, 512 * m2:512 * m2 + 512], wo_ps[:])
                    nc.sync.dma_start(
                        out[512 * b + 128 * st:512 * b + 128 * st + 128, :],
                        osb[:])
                for st in range(4):
                    pieces.append(lambda st=st: st_piece(st))
                return pieces

            # ---- pipeline ----
            import os
            _ph = int(os.environ.get("KPHASES", "3"))
            # phase 1: proj(b0) alone
            with (
                tc.tile_pool(name="xchA", bufs=2) as xpA,
                tc.tile_pool(name="p1A", bufs=2, space="PSUM") as ps1A,
                tc.tile_pool(name="vpsA", bufs=2, space="PSUM") as psvA,
            ):
                for sc in range(NJ):
                    proj_chunk(0, sc, xpA, ps1A, psvA)
            if _ph >= 2:
                # phase 2: attn(b0) interleaved with proj(b1)
                with (
                    tc.tile_pool(name="scB", bufs=2, space="PSUM") as scpB,
                    tc.tile_pool(name="avB", bufs=1, space="PSUM") as avpB,
                    tc.tile_pool(name="ptB", bufs=3) as ptpB,
                    tc.tile_pool(name="cxB", bufs=4) as cxpB,
                    tc.tile_pool(name="xchB", bufs=2) as xpB,
                    tc.tile_pool(name="p1B", bufs=1, space="PSUM") as ps1B,
                    tc.tile_pool(name="vpsB", bufs=1, space="PSUM") as psvB,
                ):
                    for j in range(NJ):
                        attn_chunk(0, j, scpB, avpB, ptpB, cxpB)
                        proj_chunk(1, j, xpB, ps1B, psvB)
                if _ph >= 3:
                    a2a(0)
                # phase 3: attn(b1) DESC interleaved with Wo(b0)
                with (
                    tc.tile_pool(name="scC", bufs=2, space="PSUM") as scpC,
                    tc.tile_pool(name="avC", bufs=1, space="PSUM") as avpC,
                    tc.tile_pool(name="ptC", bufs=3) as ptpC,
                    tc.tile_pool(name="cxC", bufs=4) as cxpC,
                    tc.tile_pool(name="wopC", bufs=1) as wopC,
                    tc.tile_pool(name="wopsC", bufs=1, space="PSUM") as wopsC,
                    tc.tile_pool(name="bcC", bufs=1, space="PSUM") as bcpsC,
                    tc.tile_pool(name="osbC", bufs=2) as osbpC,
                ):
                    pieces = (wo_pieces(0, wopC, wopsC, bcpsC, osbpC)
                              if _ph >= 3 else [])
                    # spread the 13 Wo(b0) pieces over the first attn chunks
                    sched = {0: [0], 1: [1, 2, 3], 2: [4, 5, 6],
                             3: [7, 8], 4: [9, 10], 5: [11, 12]}
                    for idx, j in enumerate(range(NJ - 1, -1, -1)):
                        attn_chunk(1, j, scpC, avpC, ptpC, cxpC)
                        if _ph >= 3:
                            for pi in sched.get(idx, []):
                                pieces[pi]()
                if _ph >= 3:
                    a2a(1)
                    # phase 4: Wo(b1)
                    with (
                        tc.tile_pool(name="wopD", bufs=1) as wopD,
                        tc.tile_pool(name="wopsD", bufs=2, space="PSUM") as wopsD,
                        tc.tile_pool(name="bcD", bufs=1, space="PSUM") as bcpsD,
                        tc.tile_pool(name="osbD", bufs=2) as osbpD,
                    ):
                        for p in wo_pieces(1, wopD, wopsD, bcpsD, osbpD):
                            p()

            rp2.release()
            qp.release()

    nc.compile()
    return nc


# ---------------------------------------------------------------------------
# Host-side sharding / assembly
# ---------------------------------------------------------------------------

def _rope_tables(token_positions, S):
    half = DHEAD // 2
    inv_freq = THETA ** (-2.0 * np.arange(half, dtype=np.float32) / DHEAD)
    angles = np.arange(4096, dtype=np.float32)[:, None] * inv_freq[None, :]
    cos_c, sin_c = np.cos(angles), np.sin(angles)
    pos = np.asarray(token_positions).astype(np.int64)
    cosT = cos_c[pos].T.astype(np.float32)   # [32, S]
    sinT = sin_c[pos].T.astype(np.float32)
    cosm = np.concatenate([cosT, cosT, cosT, cosT], 0)
    sinm = np.concatenate([sinT, -sinT, sinT, -sinT], 0)
    return (np.ascontiguousarray(cosm).astype(np_bf16),
            np.ascontiguousarray(sinm).astype(np_bf16))


def prepare_in_maps(in_features, token_positions, Wq, Wk, Wv, Wo):
    Bb, S, D = in_features.shape
    xt = np.ascontiguousarray(in_features.transpose(0, 2, 1)).astype(np_bf16)
    cosm, sinm = _rope_tables(token_positions, S)
    sel2 = np.zeros((2, 128), np.float32)
    sel2[0, :64] = 1.0
    sel2[1, 64:] = 1.0
    sel2 = sel2.astype(np_bf16)
    pp_, ss_, qq_ = np.arange(128)[:, None, None], np.arange(8)[None, :, None], \
        np.arange(512)[None, None, :]
    trimask = (qq_ >= pp_ + 128 * (ss_ // 2)).astype(np_bf16)
    sel16 = np.zeros((16, 8, 128), np.float32)
    for t_ in range(8):
        sel16[2 * t_, t_, 0:64] = 1.0
        sel16[2 * t_ + 1, t_, 64:128] = 1.0
    sel16 = sel16.astype(np_bf16)
    perm = np.concatenate([np.arange(0, 64, 2), np.arange(1, 64, 2)])
    woT = np.ascontiguousarray(Wo.T).astype(np_bf16)
    in_maps = []
    for c in range(N_CORES):
        h0, h1 = 2 * c, 2 * c + 1
        blocks_qk = []
        for W in (Wq, Wk):
            cols = []
            for h in (h0, h1):
                blk = W[64 * h:64 * h + 64, :][perm, :]   # [64, D] permuted
                cols.append(blk.T)                         # [D, 64]
            blocks_qk.append(np.ascontiguousarray(
                np.concatenate(cols, axis=1)).astype(np_bf16))
        wv_c = np.ascontiguousarray(np.concatenate(
            [Wv[64 * h:64 * h + 64, :].T for h in (h0, h1)],
            axis=1)).astype(np_bf16)
        in_maps.append({
            "xt": xt, "wq": blocks_qk[0], "wk": blocks_qk[1], "wv": wv_c,
            "wo": woT, "cosm": cosm, "sinm": sinm, "sel2": sel2,
            "trimask": trimask, "sel16": sel16,
        })
    return in_maps


def assemble(results, S):
    CH = S // 8
    out = np.zeros((B, S, D_MODEL), np.float32)
    for c in range(N_CORES):
        r = results[c]["out"]
        out[0, CH * c:CH * (c + 1), :] = r[0:CH]
        out[1, CH * c:CH * (c + 1), :] = r[CH:2 * CH]
    return out


from concourse.bass_utils import run_bass_kernel_spmd

_S = 4096
_NC = None


def _get_nc():
    global _NC
    if _NC is None:
        _NC = build_nc(_S)
    return _NC


def kernel(in_features, token_positions, Wq, Wk, Wv, Wo):
    x = np.asarray(in_features, dtype=np.float32)
    pos = np.asarray(token_positions)
    Wq = np.asarray(Wq, dtype=np.float32)
    Wk = np.asarray(Wk, dtype=np.float32)
    Wv = np.asarray(Wv, dtype=np.float32)
    Wo = np.asarray(Wo, dtype=np.float32)
    nc = _get_nc()
    in_maps = prepare_in_maps(x, pos, Wq, Wk, Wv, Wo)
    res = run_bass_kernel_spmd(nc, in_maps, list(range(N_CORES)))
    return assemble(res.results, _S)
